# revision 1
# baseline (speedup 1.0000x reference)
"""Causal self-attention (B=2, T=2048, C=1024, 16 heads) on 8 Trainium2 cores.

Sharding: core = b*4 + g. Each core handles batch b and heads [4g, 4g+4)
(256 of the 1024 channel dims). It computes q/k/v for its heads, causal
flash-style attention, and the c_proj partial product against the matching
256-row slice of w_proj. The host sums the 4 per-core partials of each batch
(equivalent to the all-reduce after c_proj, done for free on the host).

Per-core bass kernel (fp32 data, matmuls in float32r = full-rate fp32):
  phase 1 (qkv):  per head-pair qT/kT [128, 2048] (d' on partitions);
                  v as [t%128, tb, hi, d|ones] per pair
  phase 2 (attn): per head-pair, per 512-wide q block: S^T tiles
                  [j=128, q<=512] on PE (2 heads row-packed in the array),
                  exp on ACT (scale=1/8 fused), causal masking of diagonal
                  tiles on DVE — diagonal tiles are column-trimmed to the
                  valid region; P^T@V with an appended ones column (M=65)
                  gives the softmax denominators for free. Normalize via a
                  K=1 ones-matmul broadcast of the denominator row plus a
                  DVE divide, spilling oT to a DRAM scratch.
  phase 3 (proj): y[t, e] = oT.T @ w_projT accumulated over the two
                  128-row d' chunks, DMA'd out per [128, 512] tile.
"""

import numpy as np

import concourse.bass as bass
import concourse.tile as tile
from concourse import bacc, mybir
from concourse.bass_utils import run_bass_kernel_spmd

B, T, C = 2, 2048, 1024
NH, HD = 16, 64
NCORES = 8
GROUPS = 4              # head-groups; cores per batch
HPC = NH // GROUPS      # 4 heads per core
DQ = HPC * HD           # 256 head-dims per core
P = 128
CCH = C // P            # 8 contraction chunks over C
QB = 512                # q-block (free dim of S^T tiles)
NQB = T // QB           # 4
NJB = T // P            # 16 j-blocks / t-blocks of 128
EB = 512                # proj output block
F32 = mybir.dt.float32

# float32r: full-rate fp32 matmul path on trn2 (vs 4 cycles/row for
# plain fp32). Flip to mybir.dt.float32 if accuracy ever demands it.
MM_DT = mybir.dt.float32r

_PROGRAM = None
LAST_RESULTS = None     # test.py reads profiling info from here


def _build_program(reps=1):
    nc = bacc.Bacc("TRN2", target_bir_lowering=False, debug=False)

    xt_d = nc.dram_tensor("xt", [C, T], MM_DT, kind="ExternalInput")
    wqk_d = nc.dram_tensor("wqk", [C, 2 * DQ], MM_DT, kind="ExternalInput")
    wv_d = nc.dram_tensor("wv", [C, DQ], MM_DT, kind="ExternalInput")
    wp_d = nc.dram_tensor("wp", [DQ, C], MM_DT, kind="ExternalInput")
    msk_d = nc.dram_tensor("msk", [P, QB], F32, kind="ExternalInput")
    y_d = nc.dram_tensor("y", [T, C], F32, kind="ExternalOutput")

    with tile.TileContext(nc) as tc:
        with (
            tc.tile_pool(name="persist", bufs=1) as persist,
            tc.tile_pool(name="work", bufs=5) as work,
            tc.tile_pool(name="work2", bufs=2) as work2,
            tc.tile_pool(name="ps_st", bufs=2, space="PSUM") as ps_st,
            tc.tile_pool(name="ps_sm", bufs=2, space="PSUM") as ps_sm,
            tc.tile_pool(name="ps_ot", bufs=2, space="PSUM") as ps_ot,
        ):
            # ---- loads (small/critical first, x chunk-interleaved) ----
            msk = persist.tile([P, QB], F32)
            nc.scalar.dma_start(msk[:], msk_d.ap())
            xT = persist.tile([P, CCH, T], MM_DT)          # x[b].T  (c, t)
            wqk = persist.tile([P, CCH, 2 * DQ], MM_DT)    # [wq.T | wk.T] slices
            wv = persist.tile([P, CCH, DQ], MM_DT)
            xt_r = xt_d.ap().rearrange("(o p) f -> p o f", p=P)
            wqk_r = wqk_d.ap().rearrange("(o p) f -> p o f", p=P)
            for c in range(CCH):
                nc.scalar.dma_start(wqk[:, c, :], wqk_r[:, c, :])
            nc.scalar.dma_start(wv[:], wv_d.ap().rearrange("(o p) f -> p o f", p=P))

            # per head-pair tensors so attention on pair 0 can start while
            # pair 1's projections are still on the PE
            qT = [persist.tile([P, T], MM_DT, tag=f"qT{i}", name=f"qT{i}") for i in range(2)]
            kT = [persist.tile([P, T], MM_DT, tag=f"kT{i}", name=f"kT{i}") for i in range(2)]
            vv = [persist.tile([P, NJB, 2, HD + 1], MM_DT, tag=f"vv{i}",
                                name=f"vv{i}") for i in range(2)]
            for i in range(2):
                nc.vector.memset(vv[i][:, :, :, HD : HD + 1].bitcast(F32), 1.0)
            ones65 = persist.tile([HD + 1, HD], MM_DT)     # K=1 bcast weights
            nc.vector.memset(ones65[:].bitcast(F32), 1.0)
            oT = [[persist.tile([P, QB], MM_DT, tag=f"oT{i}_{q}", name=f"oT{i}_{q}")
                   for q in range(NQB)] for i in range(2)]
            wp = persist.tile([P, 2, C], MM_DT)
            nc.scalar.dma_start(wp[:], wp_d.ap().rearrange("(c p) e -> p c e", p=P))

            # ---- phase 1: qkv projections, streamed by 512-col t-blocks ----
            for _rep in range(reps):
              for tb in range(NQB):
                  # x columns for this t-block (chunked on the first block so
                  # the very first matmuls only wait for ~512KB)
                  if tb == 0:
                      for c in range(CCH):
                          nc.sync.dma_start(
                              xT[:, c, tb * QB : (tb + 1) * QB],
                              xt_r[:, c, tb * QB : (tb + 1) * QB],
                          )
                  else:
                      nc.sync.dma_start(
                          xT[:, :, tb * QB : (tb + 1) * QB],
                          xt_r[:, :, tb * QB : (tb + 1) * QB],
                      )
                  for pc in range(2):
                      ps_qk = ps_st.tile([P, 2, QB], F32, tag="st")
                      for c in range(CCH):
                          fl = dict(start=(c == 0), stop=(c == CCH - 1))
                          nc.tensor.matmul(
                              ps_qk[:, 0, :],
                              wqk[:, c, pc * P : (pc + 1) * P],
                              xT[:, c, tb * QB : (tb + 1) * QB],
                              **fl,
                          )
                          nc.tensor.matmul(
                              ps_qk[:, 1, :],
                              wqk[:, c, DQ + pc * P : DQ + (pc + 1) * P],
                              xT[:, c, tb * QB : (tb + 1) * QB],
                              **fl,
                          )
                      nc.scalar.copy(qT[pc][:, tb * QB : (tb + 1) * QB], ps_qk[:, 0, :])
                      nc.scalar.copy(kT[pc][:, tb * QB : (tb + 1) * QB], ps_qk[:, 1, :])
                  for th in range(2):      # v: 2 t-rows per 1-bank slot
                      ps_v = ps_sm.tile([P, 2, DQ], F32, tag="sm")
                      for tj in range(2):
                          tt = 4 * tb + 2 * th + tj
                          for c in range(CCH):
                              nc.tensor.matmul(
                                  ps_v[:, tj, :],
                                  xT[:, c, tt * P : (tt + 1) * P],
                                  wv[:, c, :],
                                  start=(c == 0),
                                  stop=(c == CCH - 1),
                              )
                      for tj in range(2):
                          tt = 4 * tb + 2 * th + tj
                          for pc in range(2):
                              nc.scalar.copy(
                                  vv[pc][:, tt, :, 0:HD],
                                  ps_v[:, tj, pc * P : (pc + 1) * P].rearrange(
                                      "p (h d) -> p h d", d=HD
                                  ),
                              )

              # ---- phase 2+3: attention with interleaved c_proj per q block ----
              for qi in range(NQB):
                  for pc in range(2):
                      njb = 4 * qi + 4
                      ot0 = ps_ot.tile([HD + 1, QB], F32, tag="ot")
                      ot1 = ps_ot.tile([HD + 1, QB], F32, tag="ot")
                      for jb in range(njb):
                          # diagonal tiles only contribute to columns >= 128t
                          t = jb - 4 * qi
                          lo = P * t if t > 0 else 0      # valid column start
                          w = QB - lo
                          st = ps_st.tile([P, 2, QB], F32, tag="st")  # 2 banks
                          # S^T = k^T.T @ q^T : 2 heads row-packed in the array
                          nc.tensor.matmul(
                              st[:, 0, lo:QB],
                              kT[pc][0:HD, jb * P : (jb + 1) * P],
                              qT[pc][0:HD, qi * QB + lo : (qi + 1) * QB],
                              start=True, stop=True,
                          )
                          nc.tensor.matmul(
                              st[:, 1, lo:QB],
                              kT[pc][HD:P, jb * P : (jb + 1) * P],
                              qT[pc][HD:P, qi * QB + lo : (qi + 1) * QB],
                              start=True, stop=True,
                          )
                          # one exp over both heads' tiles (amortize ACT setup)
                          e = work.tile([P, 2, QB], MM_DT, tag="e")
                          nc.scalar.activation(
                              e[:, :, lo:QB], st[:, :, lo:QB],
                              mybir.ActivationFunctionType.Exp, scale=0.125,
                          )
                          if t >= 0:
                              # causal mask: only the 128-wide diagonal band is
                              # partial; columns beyond lo+128 are fully valid
                              nc.vector.tensor_mul(
                                  e[:, :, lo : lo + P], e[:, :, lo : lo + P],
                                  msk[:, None, 0:P].to_broadcast((P, 2, P)),
                              )
                          flags = dict(start=(jb == 0), stop=(jb == njb - 1))
                          nc.tensor.matmul(
                              ot0[:, lo:QB], vv[pc][:, jb, 0, :], e[:, 0, lo:QB],
                              **flags
                          )
                          nc.tensor.matmul(
                              ot1[:, lo:QB], vv[pc][:, jb, 1, :], e[:, 1, lo:QB],
                              **flags
                          )
                      # normalize by the ones-column denominators -> oT in SBUF
                      for hi, ot in enumerate((ot0, ot1)):
                          # free the PSUM accumulator with one copy
                          osb = work2.tile([HD + 1, QB], F32, tag="osb")
                          if hi == 0:
                              nc.vector.tensor_copy(osb[:], ot[:])
                          else:
                              nc.scalar.copy(osb[:], ot[:])
                          rcr = work2.tile([HD + 1, QB], MM_DT, tag="rcr")
                          nc.vector.tensor_copy(
                              rcr[HD : HD + 1, :], osb[HD : HD + 1, :]
                          )
                          # broadcast denom across partitions via a K=1 matmul,
                          # then one reciprocal straight off the PSUM result
                          bc = ps_sm.tile([HD, QB], F32, tag="sm")
                          nc.tensor.matmul(
                              bc[:], ones65[HD : HD + 1, :], rcr[HD : HD + 1, :],
                              start=True, stop=True,
                          )
                          bcs = work2.tile([HD, QB], F32, tag="bcs")
                          nc.vector.reciprocal(bcs[:], bc[:])
                          nc.vector.tensor_mul(
                              oT[pc][qi][hi * HD : (hi + 1) * HD, :],
                              osb[0:HD, :], bcs[:],
                          )

                  # c_proj for this q block: fills PE gaps, spreads the y DMA
                  for ti in range(4):
                      tt = 4 * qi + ti
                      for eb in range(C // EB):
                          yp = ps_sm.tile([P, EB], F32, tag="sm")
                          for pc in range(2):
                              nc.tensor.matmul(
                                  yp[:],
                                  oT[pc][qi][:, ti * P : (ti + 1) * P],
                                  wp[:, pc, eb * EB : (eb + 1) * EB],
                                  start=(pc == 0),
                                  stop=(pc == 1),
                              )
                          ys = work.tile([P, EB], F32, tag="ys")
                          nc.vector.tensor_copy(ys[:], yp[:])
                          nc.sync.dma_start(
                              y_d.ap()[tt * P : (tt + 1) * P,
                                       eb * EB : (eb + 1) * EB],
                              ys[:],
                          )

    nc.compile()
    return nc


def _get_program():
    global _PROGRAM
    if _PROGRAM is None:
        import os
        _PROGRAM = _build_program(reps=int(os.environ.get("KERNEL_REPS", "1")))
    return _PROGRAM


def _masks():
    # mask[p, f] = 1 where f >= p; diagonal tile t uses columns [0, QB-128t)
    # of this against e[:, 128t:QB] (the pattern is shift-invariant).
    f = np.arange(QB)[None, :]
    p = np.arange(P)[:, None]
    return (f >= p).astype(np.float32)


def make_in_maps(x, w_qkv, w_proj):
    x = np.asarray(x, dtype=np.float32)
    w_qkv = np.asarray(w_qkv, dtype=np.float32)
    w_proj = np.asarray(w_proj, dtype=np.float32)
    wq, wk, wv = w_qkv[0:C], w_qkv[C : 2 * C], w_qkv[2 * C : 3 * C]
    msk = _masks()
    xTs = [np.ascontiguousarray(x[b].T) for b in range(B)]
    in_maps = []
    for core in range(NCORES):
        b, g = divmod(core, GROUPS)
        ds = slice(g * DQ, (g + 1) * DQ)
        in_maps.append(
            {
                "xt": xTs[b],
                "wqk": np.ascontiguousarray(
                    np.concatenate([wq[ds].T, wk[ds].T], axis=1)
                ),
                "wv": np.ascontiguousarray(wv[ds].T),
                "wp": np.ascontiguousarray(w_proj[:, ds].T),
                "msk": msk,
            }
        )
    return in_maps


def kernel(x, w_qkv, w_proj):
    global LAST_RESULTS
    import os

    in_maps = make_in_maps(x, w_qkv, w_proj)
    nc = _get_program()
    try:
        res = run_bass_kernel_spmd(
            nc,
            in_maps,
            core_ids=list(range(NCORES)),
            trace=bool(os.environ.get("BASS_TRACE")),
        )
    except ModuleNotFoundError:
        # profiling hook unavailable in this environment; rerun untraced
        os.environ["BASS_NEVER_TRACE"] = "1"
        res = run_bass_kernel_spmd(nc, in_maps, core_ids=list(range(NCORES)))
    LAST_RESULTS = res
    out = np.zeros((B, T, C), dtype=np.float32)
    for core in range(NCORES):
        out[core // GROUPS] += res.results[core]["y"]
    return out



# revision 67
# speedup vs baseline: 1.2525x; 1.2525x over previous
"""Causal self-attention (B=2, T=2048, C=1024, 16 heads) on 8 Trainium2 cores.

Sharding: core = b*4 + g. Each core handles batch b and heads [4g, 4g+4)
(256 of the 1024 channel dims). It computes q/k/v for its heads, causal
flash-style attention, and the c_proj partial product against the matching
256-row slice of w_proj. The host sums the 4 per-core partials of each batch
(the all-reduce after c_proj, done for free on the host).

v2 layout (all SBUF data bf16, PSUM accumulation fp32):
  The four q-block iterations are fully software-pipelined: the attention
  steps for q-block qi are interleaved (at instruction granularity) with the
  qkv projections for t-block qi+1 and the c_proj matmuls for q-block qi-1,
  so the PE never idles while the ACT engine works through the exp stream.
  PSUM->SBUF copies run on Pool (gpsimd) and DVE, keeping ACT exp-only.
  The softmax denominator (ones column appended to v, M=65) is broadcast
  across partitions with gpsimd.partition_broadcast instead of a PE matmul.
"""

import numpy as np

import concourse.bass as bass
import concourse.tile as tile
from concourse import bacc, mybir
from concourse.bass_utils import run_bass_kernel_spmd

B, T, C = 2, 2048, 1024
NH, HD = 16, 64
NCORES = 8
GROUPS = 4              # head-groups; cores per batch
HPC = NH // GROUPS      # 4 heads per core
DQ = HPC * HD           # 256 head-dims per core
P = 128
CCH = C // P            # 8 contraction chunks over C
QB = 512                # q-block (free dim of S^T tiles)
NQB = T // QB           # 4
NJB = T // P            # 16 j-blocks / t-blocks of 128
EB = 512                # proj output block
F32 = mybir.dt.float32
BF = mybir.dt.bfloat16

_PROGRAM = None
LAST_RESULTS = None     # test.py reads profiling info from here


def _build_program(reps=1):
    nc = bacc.Bacc("TRN2", target_bir_lowering=False, debug=False)

    xt_d = nc.dram_tensor("xt", [C, T], BF, kind="ExternalInput")
    wqk_d = nc.dram_tensor("wqk", [C, 2 * DQ], BF, kind="ExternalInput")
    wv_d = nc.dram_tensor("wv", [C, DQ], BF, kind="ExternalInput")
    wp_d = nc.dram_tensor("wp", [DQ, C], BF, kind="ExternalInput")
    msk_d = nc.dram_tensor("msk", [P, P], BF, kind="ExternalInput")
    y_d = nc.dram_tensor("y", [T, C], BF, kind="ExternalOutput")

    with tile.TileContext(nc) as tc:
        with (
            tc.tile_pool(name="persist", bufs=1) as persist,
            tc.tile_pool(name="ework", bufs=6) as ework,
            tc.tile_pool(name="nwork", bufs=4) as nwork,
            tc.tile_pool(name="ywork", bufs=4) as ywork,
            tc.tile_pool(name="ps_st", bufs=2, space="PSUM") as ps_st,   # 2x4KB
            tc.tile_pool(name="ps_ot", bufs=2, space="PSUM") as ps_ot,   # 2x2KB
            tc.tile_pool(name="ps_sm", bufs=2, space="PSUM") as ps_sm,   # 2x2KB
        ):
            # ---- persistent SBUF tiles ----
            msk = persist.tile([P, P], BF)
            xT = persist.tile([P, CCH, T], BF)             # x[b].T  (c, t)
            wqk = persist.tile([P, CCH, 2 * DQ], BF)       # [wq.T | wk.T] slices
            wv = persist.tile([P, CCH, DQ], BF)
            wp = persist.tile([P, 2, C], BF)
            qT = [persist.tile([P, T], BF, tag=f"qT{i}", name=f"qT{i}") for i in range(2)]
            kT = [persist.tile([P, T], BF, tag=f"kT{i}", name=f"kT{i}") for i in range(2)]
            vv = [persist.tile([P, NJB, 2, HD + 1], BF, tag=f"vv{i}",
                               name=f"vv{i}") for i in range(2)]
            oT = [[persist.tile([P, QB], BF, tag=f"oT{i}_{q}", name=f"oT{i}_{q}")
                   for q in range(NQB)] for i in range(2)]

            ones = persist.tile([1, HD], mybir.dt.float32r)
            nc.vector.memset(ones[:].bitcast(F32), 1.0)
            oneb = persist.tile([1, HD], BF)
            nc.vector.memset(oneb[:], 1.0)

            # ---- header DMAs, ordered for earliest possible first matmul.
            # Weight/x slices keep >=512B contiguous runs (full DMA rate):
            # x first half, then weights in 2-chunk row pieces, streaming.
            xt_r = xt_d.ap().rearrange("(o p) f -> p o f", p=P)
            wqk_r = wqk_d.ap().rearrange("(o p) f -> p o f", p=P)
            HB = QB // 2
            nc.scalar.dma_start(wqk[:, 0:2, :], wqk_r[:, 0:2, :])
            nc.sync.dma_start(xT[:, 0:4, 0:HB], xt_r[:, 0:4, 0:HB])
            nc.scalar.dma_start(wqk[:, 2:4, :], wqk_r[:, 2:4, :])
            nc.sync.dma_start(xT[:, 4:CCH, 0:HB], xt_r[:, 4:CCH, 0:HB])
            nc.scalar.dma_start(wqk[:, 4:6, :], wqk_r[:, 4:6, :])
            nc.scalar.dma_start(wqk[:, 6:CCH, :], wqk_r[:, 6:CCH, :])
            nc.sync.dma_start(xT[:, 0:4, HB:QB], xt_r[:, 0:4, HB:QB])
            nc.sync.dma_start(xT[:, 4:CCH, HB:QB], xt_r[:, 4:CCH, HB:QB])
            nc.scalar.dma_start(msk[:], msk_d.ap())
            nc.scalar.dma_start(wv[:], wv_d.ap().rearrange("(o p) f -> p o f", p=P))
            nc.scalar.dma_start(wp[:], wp_d.ap().rearrange("(c p) e -> p c e", p=P))
            for tb in range(1, NQB):
                nc.sync.dma_start(
                    xT[:, :, tb * QB : (tb + 1) * QB],
                    xt_r[:, :, tb * QB : (tb + 1) * QB],
                )
            for i in range(2):
                nc.vector.memset(vv[i][:, :, :, HD : HD + 1], 1.0)

            # warm-up matmuls on the ones tile while the header DMAs land:
            # keeps pe_busy_start early so real matmuls start at full clock
            wrm = ps_ot.tile([HD, HD], F32, tag="ot", name="wrm")
            for _ in range(32):
                nc.tensor.matmul(wrm[:], ones[0:1, :], ones[0:1, :],
                                 start=True, stop=True)

            for _rep in range(reps):

                def gen_qkv(tb, part="all"):
                    """Generator emitting qkv projections for t-block tb in
                    ~0.4us PE chunks. tb==0 streams in 256-col pieces against
                    the header DMAs (using the idle ps_st pool)."""
                    t0, t1 = tb * QB, (tb + 1) * QB
                    if part == "v":
                        pass
                    elif tb == 0:
                        # chunk-pair-major against the streaming header DMAs
                        stq = ps_st.tile([P, 2, QB], F32, tag="st", name="stq")
                        stk = ps_st.tile([P, 2, QB], F32, tag="st", name="stk")
                        for pi in range(2):
                            sl = slice(pi * HB, pi * HB + HB)
                            for cp in range(4):
                                for which, stt in ((0, stq), (1, stk)):
                                    for pc in range(2):
                                        off = which * DQ + pc * P
                                        for c in (2 * cp, 2 * cp + 1):
                                            nc.tensor.matmul(
                                                stt[:, pc, sl],
                                                wqk[:, c, off : off + P],
                                                xT[:, c, pi * HB : pi * HB + HB],
                                                start=(c == 0),
                                                stop=(c == CCH - 1),
                                            )
                                    yield
                        for which, stt in ((0, stq), (1, stk)):
                            for pc in range(2):
                                nc.vector.tensor_copy(
                                    (qT, kT)[which][pc][:, t0:t1], stt[:, pc, :]
                                )
                                yield
                    else:
                        for pc in range(2):
                            for which in range(2):      # 0 = q, 1 = k
                                dst = (qT, kT)[which][pc]
                                off = which * DQ + pc * P
                                ps_q = ps_sm.tile([P, QB], F32, tag="sm")
                                for c in range(CCH):
                                    nc.tensor.matmul(
                                        ps_q[:],
                                        wqk[:, c, off : off + P],
                                        xT[:, c, t0:t1],
                                        start=(c == 0),
                                        stop=(c == CCH - 1),
                                    )
                                    if c % 2 == 1:
                                        yield
                                nc.vector.tensor_copy(dst[:, t0:t1], ps_q[:])
                                yield
                    if part == "qk":
                        return
                    for th in range(2):             # v: 2 t-rows per slot
                        ps_v = ps_sm.tile([P, 2, DQ], F32, tag="sm")
                        for tj in range(2):
                            tt = 4 * tb + 2 * th + tj
                            for c in range(CCH):
                                nc.tensor.matmul(
                                    ps_v[:, tj, :],
                                    xT[:, c, tt * P : (tt + 1) * P],
                                    wv[:, c, :],
                                    start=(c == 0),
                                    stop=(c == CCH - 1),
                                )
                                if c % 4 == 3:
                                    yield
                        for tj in range(2):
                            tt = 4 * tb + 2 * th + tj
                            for pc in range(2):
                                nc.vector.tensor_copy(
                                    vv[pc][:, tt, :, 0:HD],
                                    ps_v[:, tj, pc * P : (pc + 1) * P].rearrange(
                                        "p (h d) -> p h d", d=HD
                                    ),
                                )
                        yield

                def gen_proj(qi, tis=(0, 1, 2, 3)):
                    """Generator emitting c_proj + y store for q-block qi.
                    One merged [P, C] store per 128-row t-block."""
                    for ti in tis:
                        tt = 4 * qi + ti
                        ys = ywork.tile([P, C], BF, tag="ys")
                        for eb in range(C // EB):
                            yp = ps_sm.tile([P, EB], F32, tag="sm")
                            for pc in range(2):
                                nc.tensor.matmul(
                                    yp[:],
                                    oT[pc][qi][:, ti * P : (ti + 1) * P],
                                    wp[:, pc, eb * EB : (eb + 1) * EB],
                                    start=(pc == 0),
                                    stop=(pc == 1),
                                )
                            yield
                            nc.vector.tensor_copy(
                                ys[:, eb * EB : (eb + 1) * EB], yp[:]
                            )
                            yield
                        nc.sync.dma_start(
                            y_d.ap()[tt * P : (tt + 1) * P, :], ys[:]
                        )

                def pump(gens, n, tail=False):
                    """Advance the filler generator list by ~n chunks.
                    Entries tagged "tail" are reserved for the tail pumps.
                    Returns the number of chunks actually emitted."""
                    done = 0
                    for _ in range(n):
                        live = [g for g in gens if tail or g[0] != "tail"]
                        while live:
                            try:
                                next(live[0][1])
                                done += 1
                                break
                            except StopIteration:
                                gens.remove(live[0])
                                live.pop(0)
                        if not live:
                            break
                    return done

                def drain_kind(gens, kind):
                    done = 0
                    for ent in [g for g in gens if g[0] == kind]:
                        for _ in ent[1]:
                            done += 1
                        gens.remove(ent)
                    return done

                # qkv for t-block 0 runs un-interleaved (nothing to overlap)
                for _ in gen_qkv(0):
                    pass

                fillers = []
                norm_pend = []  # deferred normalize emitters (emit into the
                                # NEXT section so PE isn't head-of-line
                                # blocked on the DVE reciprocal chain)
                avail = [0]     # estimated filler chunks remaining in deque
                EST = {"proj": 16, "qkv": 30, "qk": 20, "v": 10}
                for qi in range(NQB):
                    if qi == 0:
                        fillers.append(("qkv", gen_qkv(1)))
                        avail[0] += EST["qkv"]
                    elif qi < NQB - 1:
                        # qk due before the next q-block; v(tb) spills into
                        # q-block tb itself (drained before its jb == 4*tb)
                        fillers.append(("qkv", gen_qkv(qi + 1, "qk")))
                        fillers.append((f"v{qi + 1}", gen_qkv(qi + 1, "v")))
                        avail[0] += EST["qk"] + EST["v"]
                    if qi == NQB - 1:
                        # all c_proj work is hoarded for the ACT-bound last
                        # q-block, where attention alone can't keep PE fed
                        for pq in range(NQB - 1):
                            fillers.append(("proj", gen_proj(pq)))
                            avail[0] += EST["proj"]
                    nsteps = 2 * (4 * qi + 4) + 2
                    step_no = [0]

                    for pc in range(2):
                        njb = 4 * qi + 4
                        ot = [None, None]
                        pend = []            # pending PV emitters (lag-2)
                        for jb in range(njb):
                            if qi >= 2 and jb == 4 * qi:
                                # steps from here need t-block qi's v: those
                                # instructions must already be in the queues
                                avail[0] -= drain_kind(fillers, f"v{qi}")
                            t = jb - 4 * qi
                            lo = P * t if t > 0 else 0
                            st = ps_st.tile([P, 2, QB], F32, tag="st")
                            nc.tensor.matmul(
                                st[:, 0, lo:QB],
                                kT[pc][0:HD, jb * P : (jb + 1) * P],
                                qT[pc][0:HD, qi * QB + lo : (qi + 1) * QB],
                                start=True, stop=True,
                            )
                            nc.tensor.matmul(
                                st[:, 1, lo:QB],
                                kT[pc][HD:P, jb * P : (jb + 1) * P],
                                qT[pc][HD:P, qi * QB + lo : (qi + 1) * QB],
                                start=True, stop=True,
                            )
                            e = ework.tile([P, 2, QB], BF, tag="e")
                            nc.scalar.activation(
                                e[:, :, lo:QB], st[:, :, lo:QB],
                                mybir.ActivationFunctionType.Exp, scale=0.125,
                            )
                            if t >= 0:
                                # causal mask: only the 128-wide diagonal band
                                nc.vector.tensor_mul(
                                    e[:, :, lo : lo + P], e[:, :, lo : lo + P],
                                    msk[:, None, :].to_broadcast((P, 2, P)),
                                )

                            # deferred normalize from the previous section,
                            # then PE filler between S(jb) and PV(jb-2);
                            # rate = remaining chunks / remaining steps
                            step_no[0] += 1
                            if jb == 3:
                                while norm_pend:
                                    norm_pend.pop(0)()
                            if jb != 0:     # no pump before the norm flush
                                left = max(1, nsteps - step_no[0])
                                k = min(3, max(0, round(avail[0] / left)))
                                if qi < NQB - 1 or avail[0] > 0:
                                    avail[0] -= pump(fillers, max(k, 1))
                            if len(pend) == 3:
                                pend.pop(0)()

                            def make_pv(jb=jb, lo=lo, e=e, ot=ot, last=(jb == njb - 1)):
                                def emit():
                                    if ot[0] is None:
                                        ot[0] = ps_ot.tile([HD + 1, QB], F32, tag="ot", name="ot0")
                                        ot[1] = ps_ot.tile([HD + 1, QB], F32, tag="ot", name="ot1")
                                    flags = dict(start=(jb == 0), stop=last)
                                    nc.tensor.matmul(
                                        ot[0][:, lo:QB], vv[pc][:, jb, 0, :],
                                        e[:, 0, lo:QB], **flags,
                                    )
                                    nc.tensor.matmul(
                                        ot[1][:, lo:QB], vv[pc][:, jb, 1, :],
                                        e[:, 1, lo:QB], **flags,
                                    )
                                return emit
                            pend.append(make_pv())
                        while pend:
                            pump(fillers, 1)
                            pend.pop(0)()   # final PVs of this (qi, pc)

                        if qi == 3 and pc == 1:
                            break       # tail handles normalize + proj

                        # normalize: denom row -> reciprocal -> K=1 matmul
                        # broadcast (PE, f32r) -> oT = ot * (1/den) in bf16.
                        # The reciprocals (DVE) start right away; the PE
                        # broadcast + mul defer into the next section so the
                        # PE queue never waits on the reciprocal chain.
                        rcps = []
                        for hi in range(2):
                            rcp = nwork.tile([1, QB], BF, tag="rcp",
                                             name="rcp")
                            with nc.allow_low_precision(
                                    reason="1/denominator in bf16 is within "
                                    "the 2e-2 output tolerance"):
                                nc.vector.reciprocal(
                                    rcp[:], ot[hi][HD : HD + 1, :]
                                )
                            rcps.append(rcp)

                        def make_norm(qi=qi, pc=pc, ot=ot, rcps=rcps):
                            def emit():
                                for hi in range(2):
                                    bc = ps_sm.tile([HD, QB], F32, tag="sm",
                                                    name="bc")
                                    nc.tensor.matmul(
                                        bc[:], oneb[0:1, :], rcps[hi][0:1, :],
                                        start=True, stop=True,
                                    )
                                    bcs = nwork.tile([HD, QB], F32,
                                                     tag="bcs", name="bcs")
                                    nc.vector.tensor_copy(bcs[:], bc[:])
                                    nc.vector.tensor_mul(
                                        oT[pc][qi][hi * HD : (hi + 1) * HD, :],
                                        ot[hi][0:HD, :], bcs[:],
                                    )
                            return emit
                        norm_pend.append(make_norm())

                    # qkv for the next t-block must be fully emitted before
                    # the next q-block's attention reads it (in-order queues)
                    avail[0] -= drain_kind(fillers, "qkv")

                # tail: (qi=3, pc=1) normalize per 256-col half, each half
                # feeding its two c_proj t-rows immediately; partials go to
                # yt1 straight from PSUM (host adds yt0 + yt1).
                for half in range(2):
                    sl = slice(half * HB, half * HB + HB)
                    for hi in range(2):
                        rcp = nwork.tile([1, HB], BF, tag="rcp",
                                         name="rcph")
                        with nc.allow_low_precision(
                                reason="1/denominator in bf16 is within "
                                "the 2e-2 output tolerance"):
                            nc.vector.reciprocal(
                                rcp[:], ot[hi][HD : HD + 1, sl]
                            )
                        bc = ps_sm.tile([HD, HB], F32, tag="sm", name="bch")
                        nc.tensor.matmul(
                            bc[:], oneb[0:1, :], rcp[0:1, :],
                            start=True, stop=True,
                        )
                        bcs = nwork.tile([HD, HB], F32, tag="bcs",
                                         name="bcsh")
                        nc.vector.tensor_copy(bcs[:], bc[:])
                        nc.vector.tensor_mul(
                            oT[1][3][hi * HD : (hi + 1) * HD, sl],
                            ot[hi][0:HD, sl], bcs[:],
                        )
                    pump(fillers, 6, tail=True)
                    for ti in (2 * half, 2 * half + 1):
                        tt = 12 + ti
                        ys = ywork.tile([P, C], BF, tag="ys")
                        for eb in range(C // EB):
                            u = 2 * ti + eb
                            pool, tg = ((ps_sm, "sm") if u % 2 == 0
                                        else (ps_st, "st"))
                            yp = pool.tile([P, EB], F32, tag=tg, name="yp3b")
                            for pcp in range(2):
                                nc.tensor.matmul(
                                    yp[:],
                                    oT[pcp][3][:, ti * P : (ti + 1) * P],
                                    wp[:, pcp, eb * EB : (eb + 1) * EB],
                                    start=(pcp == 0),
                                    stop=(pcp == 1),
                                )
                            # PSUM->SBUF conversion spread over ACT/DVE/Pool
                            eng = (nc.scalar, nc.vector, nc.scalar,
                                   nc.vector, nc.scalar, nc.vector,
                                   nc.scalar, nc.vector)[u]
                            if eng is nc.scalar:
                                eng.copy(ys[:, eb * EB : (eb + 1) * EB], yp[:])
                            else:
                                eng.tensor_copy(
                                    ys[:, eb * EB : (eb + 1) * EB], yp[:]
                                )
                        nc.sync.dma_start(
                            y_d.ap()[tt * P : (tt + 1) * P, :], ys[:]
                        )

    nc.compile()
    return nc


def _get_program():
    global _PROGRAM
    if _PROGRAM is None:
        import os
        _PROGRAM = _build_program(reps=int(os.environ.get("KERNEL_REPS", "1")))
    return _PROGRAM


def _masks():
    # mask[p, f] = 1 where f >= p (triangle for the 128-wide diagonal band)
    f = np.arange(P)[None, :]
    p = np.arange(P)[:, None]
    return (f >= p).astype(np.float32)


def make_in_maps(x, w_qkv, w_proj):
    import ml_dtypes

    bf16 = ml_dtypes.bfloat16
    x = np.asarray(x, dtype=np.float32)
    w_qkv = np.asarray(w_qkv, dtype=np.float32)
    w_proj = np.asarray(w_proj, dtype=np.float32)
    wq, wk, wv = w_qkv[0:C], w_qkv[C : 2 * C], w_qkv[2 * C : 3 * C]
    msk = _masks().astype(bf16)
    xTs = [np.ascontiguousarray(x[b].T).astype(bf16) for b in range(B)]
    in_maps = []
    for core in range(NCORES):
        b, g = divmod(core, GROUPS)
        ds = slice(g * DQ, (g + 1) * DQ)
        in_maps.append(
            {
                "xt": xTs[b],
                "wqk": np.ascontiguousarray(
                    np.concatenate([wq[ds].T, wk[ds].T], axis=1)
                ).astype(bf16),
                "wv": np.ascontiguousarray(wv[ds].T).astype(bf16),
                "wp": np.ascontiguousarray(w_proj[:, ds].T).astype(bf16),
                "msk": msk,
            }
        )
    return in_maps


def kernel(x, w_qkv, w_proj):
    global LAST_RESULTS
    import os

    in_maps = make_in_maps(x, w_qkv, w_proj)
    nc = _get_program()
    try:
        res = run_bass_kernel_spmd(
            nc,
            in_maps,
            core_ids=list(range(NCORES)),
            trace=bool(os.environ.get("BASS_TRACE")),
        )
    except ModuleNotFoundError:
        # profiling hook unavailable in this environment; rerun untraced
        os.environ["BASS_NEVER_TRACE"] = "1"
        res = run_bass_kernel_spmd(nc, in_maps, core_ids=list(range(NCORES)))
    LAST_RESULTS = res
    out = np.zeros((B, T, C), dtype=np.float32)
    for core in range(NCORES):
        out[core // GROUPS] += np.asarray(res.results[core]["y"], dtype=np.float32)
    return out
